# revision 1
# baseline (speedup 1.0000x reference)
"""Causal self-attention Bass/TRN2 kernel (v2, bf16 compute).

Shapes (hardcoded): query [2, 2048, 1024], 16 heads, d=64.
Sharding: 8 cores = 2 batches x 4 head-groups (4 heads per core, tensor
parallel on the QKV/proj weight columns). Each core computes a partial
out projection out_t = Wp_slice^T @ y^T (shape [1024, 2048] f32); host sums
the 4 partials per batch, transposes, and adds the folded bias.

Host-side exact simplifications:
  * x is pre-transposed per batch (x^T [1024, 2048]) and cast to bf16, so the
    device never runs PE transposes.
  * bk is dropped: q . bk is constant along the softmax axis (shift
    invariance), so it never affects the output.
  * bv is folded into the output bias: y = P@(x Wv) + (P@1) bv^T and softmax
    rows sum to 1 after normalization, so out += bv @ Wp, added to bp on host.
  * bq is applied on device (fused into the Q PSUM->SBUF activation copy).

Per-core pipeline (all matmuls bf16, f32 PSUM accumulate):
  B(m,g): Q^T/K^T [128, 512]-chunk projections (8 k-matmuls each) + ACT copy
          (Q with bq bias) -> qt/kt bf16 [128, 2, 2048].
  C(it):  V natural [128, 256] (8 k-matmuls) + DVE copy into va bf16
          [128, h, it, 65]; column 64 is memset to 1 so the M=65 PV matmul
          also produces the softmax denominator row.
  D(hp,g): per 128-row K-block j: S^T for both heads of the pair into one
          [128, 1024] PSUM tile; additive -1e30 causal mask on the diagonal
          128-blocks (DVE); ACT exp (scale=1/8, no max subtraction -- scores
          are bounded for this problem) -> p12 bf16; PV accumulate into
          yd1/yd2 [65, 512] PSUM. Emission is software-pipelined 2 blocks
          ahead so ACT exp latency never stalls the PE. Normalization:
          DVE reciprocal of the denominator row, Pool partition_broadcast,
          DVE multiply -> yt bf16 (head 1 of the pair lands at partitions
          64:128 via a small SBUF->SBUF shift DMA).
  E(g):   out_t chunk = Wp^T y^T, staged PSUM->SBUF on alternating DVE/ACT,
          DMA out f32.
Schedule interleaves B/C/E between D chunks to keep the PE saturated while
the ACT engine drains the exp backlog.

This walrus build accepts only ONE sync-wait command per TPB instruction, so
after Tile scheduling we hoist excess waits into standalone InstEventSemaphore
instructions (split_excess_waits).
"""

import numpy as np
import ml_dtypes

import concourse.bass as bass
import concourse.mybir as mybir
import concourse.tile as tile
from concourse.bass_utils import run_bass_kernel_spmd

B, T, C, H = 2, 2048, 1024, 16
D = C // H            # 64 head dim
HC = 4                # heads per core
DC = HC * D           # 256 dcols per core
KT = C // 128         # 8 contraction tiles
NT = T // 128         # 16 t-tiles
TCH = T // 512        # 4 t-chunks of 512
SCALE = 1.0 / np.sqrt(D)
FILL_EVERY = 4
NEG = -1.0e30

f32 = mybir.dt.float32
f32r = mybir.dt.float32r
bf16 = mybir.dt.bfloat16
BF = ml_dtypes.bfloat16

_CACHE = {}


def _split_excess_waits(nc, max_inline=1):
    """Hoist excess per-instruction waits into standalone event-sem waits."""
    n = 0
    for f in nc.m.functions:
        for bb in f.blocks:
            new_insts = []
            for inst in bb.instructions:
                si = inst.sync_info
                waits = list(si.on_wait) if (si is not None and si.on_wait) else []
                if len(waits) > max_inline:
                    hoist, keep = waits[:-max_inline], waits[-max_inline:]
                    for w in hoist:
                        ev = mybir.InstEventSemaphore(
                            name=nc.get_next_instruction_name(),
                            engine=inst.engine,
                            ins=[],
                            outs=[],
                            sync_info=mybir.SyncInfo(on_wait=[w], on_update=[]),
                        )
                        nc.register_instruction(ev, overwrite=True)
                        new_insts.append(ev)
                        n += 1
                    si.on_wait = keep
                new_insts.append(inst)
            bb.instructions[:] = new_insts
    return n


def _make_diag_mask(nc, mask):
    """mask[p, f] = 0 where f >= p (valid, t>=s) else -1e30."""
    nc.gpsimd.memset(mask, 0.0)
    nc.gpsimd.affine_select(
        out=mask, in_=mask, compare_op=mybir.AluOpType.is_ge,
        fill=NEG, base=0, pattern=[[1, 128]], channel_multiplier=-1,
    )


def _build_program():
    nc = bass.Bass("TRN2", target_bir_lowering=False, debug=False)

    xt_d = nc.dram_tensor("xt", [C, T], bf16, kind="ExternalInput").ap()
    wq_d = nc.dram_tensor("wq", [C, DC], bf16, kind="ExternalInput").ap()
    wk_d = nc.dram_tensor("wk", [C, DC], bf16, kind="ExternalInput").ap()
    wv_d = nc.dram_tensor("wv", [C, DC], bf16, kind="ExternalInput").ap()
    wp_d = nc.dram_tensor("wp", [DC, C], bf16, kind="ExternalInput").ap()
    bq_d = nc.dram_tensor("bq", [DC], f32, kind="ExternalInput").ap()
    out_d = nc.dram_tensor("out_t", [C, T], bf16, kind="ExternalOutput").ap()

    ident_fn = mybir.ActivationFunctionType.Identity

    with (
        tile.TileContext(nc) as tc,
        nc.allow_low_precision("bf16 compute fits the 2e-2 rel tolerance"),
    ):
        with (
            tc.tile_pool(name="const", bufs=1) as cpool,
            tc.tile_pool(name="big", bufs=1) as big,
            tc.tile_pool(name="pp", bufs=12) as pp,
            tc.tile_pool(name="rp", bufs=4) as rp,
            tc.tile_pool(name="rbp", bufs=4) as rbp,
            tc.tile_pool(name="ytp", bufs=4) as ytp,
            tc.tile_pool(name="obp", bufs=8) as obp,
            tc.tile_pool(name="ps_ay", bufs=2, space="PSUM") as ps_ay,
            tc.tile_pool(name="ps_y", bufs=2, space="PSUM") as ps_y,
            tc.tile_pool(name="ps_s", bufs=2, space="PSUM") as ps_s,
        ):
            # PSUM budget (8 banks): acc rotation 2 (qp/kp/vp/op),
            # yd rotation 2 (yd1+yd2, decoupled so the normalize chain never
            # blocks projection tiles), s12 2 x 2 banks.
            def acc_tile():
                return ps_ay.tile([128, 512], f32, name="acc")

            def yd_tile():
                return ps_y.tile([128, 512], f32, name="yd")
            # ---- constants ----
            # tri01[p, f] = 1 where f >= p (valid, t>=s) else 0; applied
            # multiplicatively to exp(S) on the Pool engine (Pool cannot
            # access PSUM, but p12 lives in SBUF)
            tri01 = cpool.tile([128, 128], bf16)
            nc.gpsimd.memset(tri01, 1.0)
            nc.gpsimd.affine_select(
                out=tri01, in_=tri01, compare_op=mybir.AluOpType.is_ge,
                fill=0.0, base=0, pattern=[[1, 128]], channel_multiplier=-1,
            )
            bq_sb = cpool.tile([128, 2, 1], f32)
            ones_f = cpool.tile([128, 64], f32)
            nc.gpsimd.memset(ones_f, 1.0)
            ones_sb = ones_f.bitcast(f32r)

            # ---- persistent big tensors ----
            xt = big.tile([128, KT, T], bf16)      # X^T
            wq_sb = big.tile([128, KT, DC], bf16)
            wk_sb = big.tile([128, KT, DC], bf16)
            wv_sb = big.tile([128, KT, DC], bf16)
            wp_sb = big.tile([128, 2, C], bf16)
            qt = big.tile([128, 2, T], bf16)       # Q^T [dcol, t]
            kt = big.tile([128, 2, T], bf16)       # K^T
            # V augmented per head: [s, 65] = [V_h | ones]; the M=65 PV matmul
            # computes y rows 0..63 and the softmax denominator row 64.
            va = big.tile([128, HC, NT, 65], bf16)
            yt = big.tile([128, 2, T], bf16)       # normalized y^T

            nc.gpsimd.memset(va[:, :, :, 64:65], 1.0)

            # ---- input DMAs, ordered for earliest PE start ----
            # weights dispatch from the ACT queue (HWDGE path, idle at
            # startup) so their DMA issue overlaps the x^T loads from SP
            nc.scalar.dma_start(
                out=wq_sb, in_=wq_d.rearrange("(k p) d -> p k d", p=128))
            nc.scalar.dma_start(
                out=bq_sb, in_=bq_d.rearrange("(m p o) -> p m o", p=128, o=1))
            nc.scalar.dma_start(
                out=wk_sb, in_=wk_d.rearrange("(k p) d -> p k d", p=128))
            nc.scalar.dma_start(
                out=wv_sb, in_=wv_d.rearrange("(k p) d -> p k d", p=128))
            nc.scalar.dma_start(
                out=wp_sb, in_=wp_d.rearrange("(m p) c -> p m c", p=128))
            for g in range(TCH):
                nc.sync.dma_start(
                    out=xt[:, :, bass.ts(g, 512)],
                    in_=xt_d[:, bass.ts(g, 512)].rearrange(
                        "(k p) t -> p k t", p=128))

            # ---- stage helpers ----
            def proj_qk(m, g):
                ts_g = bass.ts(g, 512)
                qp = acc_tile()
                for k in range(KT):
                    nc.tensor.matmul(
                        qp, wq_sb[:, k, bass.ts(m, 128)], xt[:, k, ts_g],
                        start=(k == 0), stop=(k == KT - 1),
                    )
                nc.scalar.activation(
                    out=qt[:, m, ts_g], in_=qp, func=ident_fn,
                    bias=bq_sb[:, m, :], scale=1.0,
                )
                kp = acc_tile()
                for k in range(KT):
                    nc.tensor.matmul(
                        kp, wk_sb[:, k, bass.ts(m, 128)], xt[:, k, ts_g],
                        start=(k == 0), stop=(k == KT - 1),
                    )
                nc.scalar.copy(out=kt[:, m, ts_g], in_=kp)

            def proj_v(it):
                # full-bank allocation (use first DC cols) to avoid
                # intra-bank PE-write / DVE-read overlap
                vp_full = acc_tile()
                vp = vp_full[:, 0:DC]
                for k in range(KT):
                    nc.tensor.matmul(
                        vp, xt[:, k, bass.ts(it, 128)], wv_sb[:, k, :],
                        start=(k == 0), stop=(k == KT - 1),
                    )
                nc.vector.tensor_copy(
                    out=va[:, :, it, 0:64],
                    in_=vp.rearrange("p (h d) -> p h d", h=HC),
                )

            def attn(hp, g, pull=None):
                nj = 4 * g + 4
                yd1 = yd_tile()
                yd2 = yd_tile()

                def emit_s(j):
                    r = j - 4 * g
                    lo = 128 * r if r > 0 else 0
                    w = 512 - lo
                    s12 = ps_s.tile([128, 1024], f32, name="s12")
                    tsl = bass.ds(512 * g + lo, w)
                    nc.tensor.matmul(
                        s12[:, lo:512], kt[0:64, hp, bass.ts(j, 128)],
                        qt[0:64, hp, tsl], start=True, stop=True,
                    )
                    nc.tensor.matmul(
                        s12[:, 512 + lo:1024], kt[64:128, hp, bass.ts(j, 128)],
                        qt[64:128, hp, tsl], start=True, stop=True,
                    )
                    p12 = pp.tile([128, 1024], bf16, name="p12")
                    sv = s12.rearrange("p (h t) -> p h t", h=2)[:, :, lo:]
                    pv_ = p12.rearrange("p (h t) -> p h t", h=2)[:, :, lo:]
                    nc.scalar.activation(
                        out=pv_, in_=sv,
                        func=mybir.ActivationFunctionType.Exp,
                        scale=float(SCALE),
                    )
                    if r >= 0:
                        # zero the upper triangle of the diagonal block after
                        # exp, on the otherwise-idle Pool engine (unmasked
                        # scores are bounded, so exp cannot overflow; the
                        # denominator is formed from the masked p12 by PV)
                        nc.gpsimd.tensor_mul(
                            p12[:, lo:lo + 128], p12[:, lo:lo + 128], tri01)
                        nc.gpsimd.tensor_mul(
                            p12[:, 512 + lo:512 + lo + 128],
                            p12[:, 512 + lo:512 + lo + 128], tri01)
                    return (j, p12, lo)

                def emit_pv(j, p12, lo):
                    last = j == nj - 1
                    nc.tensor.matmul(
                        yd1[0:65, lo:], va[:, (2 * hp) % 4, j, :],
                        p12[:, lo:512], start=(j == 0), stop=last,
                        skip_group_check=True,
                    )
                    nc.tensor.matmul(
                        yd2[0:65, lo:], va[:, (2 * hp + 1) % 4, j, :],
                        p12[:, 512 + lo:1024], start=(j == 0), stop=last,
                        skip_group_check=True,
                    )

                # software pipeline: PV lags S/exp by 2 blocks so ACT exp
                # latency stays off the PE critical path; filler units
                # (projection/out-proj pieces) keep the PE busy while ACT
                # drains the exp backlog
                pend = []
                for j in range(nj):
                    pend.append(emit_s(j))
                    if len(pend) > 2:
                        emit_pv(*pend.pop(0))
                    if pull is not None and j % FILL_EVERY == FILL_EVERY - 1:
                        pull(2 if (hp == 1 and g >= TCH - 2) else 1)
                while pend:
                    emit_pv(*pend.pop(0))

                # normalize: 1/den (row 64), broadcast to 64 rows via a
                # K=1 ones matmul at row group (64,0) (walrus rejects
                # InstPartitionBroadcast), multiply into yt. The broadcast
                # PSUM tile borrows the s12 rotation slot.
                for head, yd in ((0, yd1), (1, yd2)):
                    r1 = rp.tile([128, 512], f32r, name="r1")
                    nc.vector.reciprocal(out=r1[64:65, :], in_=yd[64:65, :])
                    bct = ps_s.tile([128, 1024], f32, name="s12")
                    bc = bct[0:64, 0:512]
                    nc.tensor.matmul(
                        bc, ones_sb[64:65, :], r1[64:65, :],
                        start=True, stop=True,
                    )
                    rb = rbp.tile([64, 512], f32, name="rb")
                    nc.vector.tensor_copy(out=rb, in_=bc)
                    if head == 0:
                        nc.vector.tensor_mul(
                            yt[0:64, hp, bass.ts(g, 512)], yd[0:64, :], rb)
                    else:
                        ytmp = ytp.tile([64, 512], bf16, name="ytmp")
                        nc.vector.tensor_mul(ytmp, yd[0:64, :], rb)
                        nc.sync.dma_start(
                            out=yt[64:128, hp, bass.ts(g, 512)], in_=ytmp)

            def unit_q(m, g):
                def emit():
                    ts_g = bass.ts(g, 512)
                    qp = acc_tile()
                    for k in range(KT):
                        nc.tensor.matmul(
                            qp, wq_sb[:, k, bass.ts(m, 128)], xt[:, k, ts_g],
                            start=(k == 0), stop=(k == KT - 1),
                        )
                    nc.scalar.activation(
                        out=qt[:, m, ts_g], in_=qp, func=ident_fn,
                        bias=bq_sb[:, m, :], scale=1.0,
                    )
                return emit

            def unit_k(m, g):
                def emit():
                    ts_g = bass.ts(g, 512)
                    kp = acc_tile()
                    for k in range(KT):
                        nc.tensor.matmul(
                            kp, wk_sb[:, k, bass.ts(m, 128)], xt[:, k, ts_g],
                            start=(k == 0), stop=(k == KT - 1),
                        )
                    nc.scalar.copy(out=kt[:, m, ts_g], in_=kp)
                return emit

            def unit_v(it):
                def emit():
                    vp_full = acc_tile()
                    vp = vp_full[:, 0:DC]
                    for k in range(KT):
                        nc.tensor.matmul(
                            vp, xt[:, k, bass.ts(it, 128)], wv_sb[:, k, :],
                            start=(k == 0), stop=(k == KT - 1),
                        )
                    nc.vector.tensor_copy(
                        out=va[:, :, it, 0:64],
                        in_=vp.rearrange("p (h d) -> p h d", h=HC),
                    )
                return emit

            def unit_e(g, mo, deep=False):
                def emit():
                    # after the last attention chunk the yd banks are free:
                    # rotating through them doubles the out-proj pipeline
                    # depth at the exposed tail
                    op = yd_tile() if (deep and mo % 2) else acc_tile()
                    for m in range(2):
                        nc.tensor.matmul(
                            op, wp_sb[:, m, bass.ts(mo, 128)],
                            yt[:, m, bass.ts(g, 512)],
                            start=(m == 0), stop=(m == 1),
                        )
                    ob = obp.tile([128, 512], bf16, name="ob")
                    if deep:
                        # tail: ACT is idle once the exp stream has drained
                        nc.scalar.copy(out=ob, in_=op)
                    elif mo % 2 == 0:
                        nc.vector.tensor_copy(out=ob, in_=op)
                    else:
                        nc.scalar.copy(out=ob, in_=op)
                    nc.sync.dma_start(
                        out=out_d[bass.ts(mo, 128), bass.ts(g, 512)], in_=ob)
                return emit

            # ---- emission schedule ----
            # Unit queue: projection / V / out-proj pieces are emitted either
            # as prerequisites before the attention chunk that needs them or
            # pulled one at a time between attention blocks as PE filler
            # while the ACT engine works through the exp stream.
            queue = []  # (label, emit_fn) in dependency-safe order

            def pull(n):
                for _ in range(n):
                    if not queue:
                        return
                    queue.pop(0)[1]()

            def drain_until(label):
                while queue:
                    lab, fn = queue.pop(0)
                    fn()
                    if lab == label:
                        return

            # prerequisites for attn(0,0) run eagerly (PE ramps while the
            # remaining input DMAs land)
            unit_q(0, 0)()
            unit_k(0, 0)()
            for it in range(4):
                unit_v(it)()

            for g in range(1, TCH):
                queue.append((f"q0{g}", unit_q(0, g)))
                queue.append((f"k0{g}", unit_k(0, g)))
                for it in range(4 * g, 4 * g + 4):
                    queue.append((f"v{it}", unit_v(it)))
            # second head-pair runs big-chunk-first so the final attention
            # chunk (and therefore the exposed out-proj tail) is the smallest
            aorder = [0, 1, 2, 3]
            for g in aorder:
                queue.append((f"q1{g}", unit_q(1, g)))
                queue.append((f"k1{g}", unit_k(1, g)))

            for g in range(TCH):
                attn(0, g, pull)
                if g < TCH - 1:
                    drain_until(f"v{4 * (g + 1) + 3}")
            for i, g in enumerate(aorder):
                drain_until(f"k1{g}")
                attn(1, g, pull)
                if i > 0:
                    gp = aorder[i - 1]
                    for mo in range(8):
                        queue.append((f"e{gp}{mo}", unit_e(gp, mo)))
            gl = aorder[-1]
            for mo in range(8):
                queue.append((f"e{gl}{mo}", unit_e(gl, mo, deep=True)))
            while queue:
                queue.pop(0)[1]()

    _split_excess_waits(nc)
    return nc


def kernel(**inputs) -> np.ndarray:
    query = np.ascontiguousarray(np.asarray(inputs["query"], dtype=np.float32))
    Wq = np.asarray(inputs["Wq"], dtype=np.float32)
    Wk = np.asarray(inputs["Wk"], dtype=np.float32)
    Wv = np.asarray(inputs["Wv"], dtype=np.float32)
    Wp = np.asarray(inputs["Wp"], dtype=np.float32)
    bq = np.asarray(inputs["bq"], dtype=np.float32)
    bk = np.asarray(inputs["bk"], dtype=np.float32)  # noqa: F841 (exactly dropped)
    bv = np.asarray(inputs["bv"], dtype=np.float32)
    bp = np.asarray(inputs["bp"], dtype=np.float32)
    n_head = int(inputs.get("n_head", H))
    assert n_head == H, f"kernel hardcodes n_head={H}, got {n_head}"
    assert query.shape == (B, T, C)

    if "nc" not in _CACHE:
        _CACHE["nc"] = _build_program()
    nc = _CACHE["nc"]

    # bv contributes bv @ Wp to every output row (softmax rows sum to 1)
    bp_eff = bp + bv @ Wp

    xt_b = [np.ascontiguousarray(query[b].T).astype(BF) for b in range(B)]
    in_maps = []
    for c in range(8):
        b, hg = divmod(c, 4)
        cols = slice(DC * hg, DC * (hg + 1))
        in_maps.append({
            "xt": xt_b[b],
            "wq": np.ascontiguousarray(Wq[:, cols]).astype(BF),
            "wk": np.ascontiguousarray(Wk[:, cols]).astype(BF),
            "wv": np.ascontiguousarray(Wv[:, cols]).astype(BF),
            "wp": np.ascontiguousarray(Wp[cols, :]).astype(BF),
            "bq": np.ascontiguousarray(bq[cols]),
        })

    res = run_bass_kernel_spmd(nc, in_maps, core_ids=list(range(8)))
    _CACHE["last_res"] = res

    out = np.empty((B, T, C), np.float32)
    for b in range(B):
        acc = np.asarray(res.results[4 * b]["out_t"], dtype=np.float32)
        for c in range(4 * b + 1, 4 * b + 4):
            acc = acc + np.asarray(res.results[c]["out_t"], dtype=np.float32)
        out[b] = acc.T + bp_eff
    return out



# revision 13
# speedup vs baseline: 1.1105x; 1.1105x over previous
"""Causal self-attention Bass/TRN2 kernel (v3).

Shapes (hardcoded): query [2, 2048, 1024], 16 heads, d=64.
Sharding: 8 cores = 2 batches x 4 head-groups (4 heads per core, tensor
parallel on the QKV/proj weight columns). Each core computes a partial
out projection out_t = Wp_slice^T @ y^T (shape [1024, 2048] bf16); host sums
the 4 partials per batch, transposes, and adds the folded bias.

Cost-model-driven structure (TimelineSim charges moving-columns only):
  * QKV projections run as 3-term residual-fp8 DoubleRow matmuls
    (x8@w8 + x8@wr + xr@w8, all e4m3, K=256 per instruction at 0.5
    cycles/row): 25% fewer PE cycles than bf16 at slightly BETTER accuracy
    (x8+xr carries ~9 mantissa bits vs bf16's 8). Weights are pre-scaled
    by 32 on host so all three terms share one PSUM scale; the PSUM->SBUF
    copy divides by 32.
  * PV uses p12 as the STATIONARY operand and the V-block [128, 65]
    (64 dims + ones column for the denominator) as MOVING: out y lands in
    natural [t, d] layout at 65 moving-columns per (j-block, t-subtile)
    instead of w columns -- ~2x cheaper than the y^T formulation.
  * The softmax denominator is then a per-partition scalar: DVE reciprocal
    [128, 4] + tensor_scalar multiply replace the reciprocal/broadcast-
    matmul/rowcopy chain of the y^T formulation.
  * y^T for the out-projection is recovered with PE transposes (128 cycles
    per [128, 128] tile; identity-permutation matmul).
  * bk dropped (softmax shift invariance); bv folded into the output bias
    on host; bq applied in the Q PSUM->SBUF copy.

Per-core pipeline:
  B(m,g): Q^T/K^T [128, 512]-chunk projections (12 DoubleRow matmuls) ->
          qt/kt bf16. Q copy on ACT (bias+1/32 scale), K on DVE.
  C(it):  V natural [128, 256] (12 DoubleRow matmuls) + DVE scaled copy into
          va bf16 [128, h, it, 65]; column 64 memset to 1.
  D(hp,g): per 128-row K-block j: S^T for both heads into one [128, 1024]
          PSUM tile; ACT exp (scale=1/8) -> p12 bf16; Pool tri01 mask on the
          diagonal blocks; PV accumulates into yd [128, 4, 65] natural tiles
          (one per t-subtile per head). Emission software-pipelined 2 blocks
          ahead; projection/transpose/out-proj units pulled as PE filler.
          Normalize: DVE reciprocal of yd[:, :, 64] + tensor_scalar -> yt_nat.
  T(hp,g): 4 PE transposes [128, 128] -> one DVE copy into ytT.
  E(g,mo): out_t chunk = Wp^T y^T (bf16), staged PSUM->SBUF on DVE/ACT,
          DMA out bf16.
Chunk order 1,2,3,0 so the final attention chunk (and its trailing exp
backlog) is the smallest.

Walrus accepts only ONE sync-wait per TPB instruction; excess waits are
hoisted into standalone InstEventSemaphore (split_excess_waits).
"""

import numpy as np
import ml_dtypes

import concourse.bass as bass
import concourse.mybir as mybir
import concourse.tile as tile
from concourse.bass_utils import run_bass_kernel_spmd

B, T, C, H = 2, 2048, 1024, 16
D = C // H            # 64 head dim
HC = 4                # heads per core
DC = HC * D           # 256 dcols per core
KT = C // 128         # 8 contraction tiles
NT = T // 128         # 16 t-tiles
TCH = T // 512        # 4 t-chunks of 512
SCALE = 1.0 / np.sqrt(D)
FILL_EVERY = 4
WS = 32.0             # host weight pre-scale for fp8

f32 = mybir.dt.float32
bf16 = mybir.dt.bfloat16
fp8 = mybir.dt.float8e4
BF = ml_dtypes.bfloat16
F8 = ml_dtypes.float8_e4m3
DR = mybir.MatmulPerfMode.DoubleRow

_CACHE = {}


def _split_excess_waits(nc, max_inline=1):
    """Hoist excess per-instruction waits into standalone event-sem waits."""
    n = 0
    for f in nc.m.functions:
        for bb in f.blocks:
            new_insts = []
            for inst in bb.instructions:
                si = inst.sync_info
                waits = list(si.on_wait) if (si is not None and si.on_wait) else []
                if len(waits) > max_inline:
                    hoist, keep = waits[:-max_inline], waits[-max_inline:]
                    for w in hoist:
                        ev = mybir.InstEventSemaphore(
                            name=nc.get_next_instruction_name(),
                            engine=inst.engine,
                            ins=[],
                            outs=[],
                            sync_info=mybir.SyncInfo(on_wait=[w], on_update=[]),
                        )
                        nc.register_instruction(ev, overwrite=True)
                        new_insts.append(ev)
                        n += 1
                    si.on_wait = keep
                new_insts.append(inst)
            bb.instructions[:] = new_insts
    return n


def _build_program(debug_dumps=False):
    nc = bass.Bass("TRN2", target_bir_lowering=False, debug=False)

    x8_d = nc.dram_tensor("x8", [C, T], fp8, kind="ExternalInput").ap()
    xr8_d = nc.dram_tensor("xr8", [C, T], fp8, kind="ExternalInput").ap()
    wq8_d = nc.dram_tensor("wq8", [C, DC], fp8, kind="ExternalInput").ap()
    wqr_d = nc.dram_tensor("wqr", [C, DC], fp8, kind="ExternalInput").ap()
    wk8_d = nc.dram_tensor("wk8", [C, DC], fp8, kind="ExternalInput").ap()
    wkr_d = nc.dram_tensor("wkr", [C, DC], fp8, kind="ExternalInput").ap()
    wv8_d = nc.dram_tensor("wv8", [C, DC], fp8, kind="ExternalInput").ap()
    wvr_d = nc.dram_tensor("wvr", [C, DC], fp8, kind="ExternalInput").ap()
    wp_d = nc.dram_tensor("wp", [DC, C], bf16, kind="ExternalInput").ap()
    bq_d = nc.dram_tensor("bq", [DC], f32, kind="ExternalInput").ap()
    out_d = nc.dram_tensor("out_t", [C, T], bf16, kind="ExternalOutput").ap()

    ident_fn = mybir.ActivationFunctionType.Identity

    def drearr(ap2d):
        return ap2d.rearrange("(k p) t -> p k t", p=128)

    with (
        tile.TileContext(nc) as tc,
        nc.allow_low_precision("fp8/bf16 compute fits the 2e-2 rel tolerance"),
    ):
        with (
            tc.tile_pool(name="const", bufs=1) as cpool,
            tc.tile_pool(name="big", bufs=1) as big,
            tc.tile_pool(name="pp", bufs=12) as pp,
            tc.tile_pool(name="rcpp", bufs=6) as rcpp,
            tc.tile_pool(name="obp", bufs=8) as obp,
            tc.tile_pool(name="ps_a", bufs=2, space="PSUM") as ps_a,
            tc.tile_pool(name="ps_s", bufs=2, space="PSUM") as ps_s,
            tc.tile_pool(name="ps_y", bufs=2, space="PSUM") as ps_y,
        ):
            # PSUM budget (8 banks): acc 2x1, s12 2x2, yd 2x1.
            def acc_tile():
                return ps_a.tile([128, 512], f32, name="acc")

            # ---- constants ----
            # tri01[p, f] = 1 where f >= p (valid, t>=s) else 0 (Pool mask)
            tri01 = cpool.tile([128, 128], bf16)
            nc.gpsimd.memset(tri01, 1.0)
            nc.gpsimd.affine_select(
                out=tri01, in_=tri01, compare_op=mybir.AluOpType.is_ge,
                fill=0.0, base=0, pattern=[[1, 128]], channel_multiplier=-1,
            )
            # identity permutation for PE transposes
            ident = cpool.tile([128, 128], bf16)
            nc.gpsimd.memset(ident, 1.0)
            nc.gpsimd.affine_select(
                out=ident, in_=ident, compare_op=mybir.AluOpType.is_equal,
                fill=0.0, base=0, pattern=[[1, 128]], channel_multiplier=-1,
            )
            bq_sb = cpool.tile([128, 2, 1], f32)

            # ---- persistent big tensors ----
            xt8 = big.tile([128, KT, T], fp8)
            xtr8 = big.tile([128, KT, T], fp8)
            wq8 = big.tile([128, KT, DC], fp8)
            wqr = big.tile([128, KT, DC], fp8)
            wk8 = big.tile([128, KT, DC], fp8)
            wkr = big.tile([128, KT, DC], fp8)
            wv8 = big.tile([128, KT, DC], fp8)
            wvr = big.tile([128, KT, DC], fp8)
            wp_sb = big.tile([128, 2, C], bf16)
            qt = big.tile([128, 2, T], bf16)       # Q^T [dcol, t]
            kt = big.tile([128, 2, T], bf16)       # K^T
            # V augmented per head: [s, 65] = [V_h | ones]
            va = big.tile([128, HC, NT, 65], bf16)
            yt_nat = big.tile([128, 2, NT, 128], bf16)  # y natural [t, dc]
            ytT = big.tile([128, 2, T], bf16)           # y^T [dc, t]

            nc.gpsimd.memset(va[:, :, :, 64:65], 1.0)

            # ---- input DMAs, ordered for earliest PE start ----
            # first attention chunk is g=1: Q(0,1) term-1 needs wq8 + xt8 g1.
            nc.sync.dma_start(out=wq8, in_=drearr(wq8_d))
            g1 = bass.ts(1, 512)
            nc.sync.dma_start(out=xt8[:, 0:2, g1], in_=drearr(x8_d[:, g1])[:, 0:2, :])
            nc.sync.dma_start(out=xt8[:, 2:KT, g1], in_=drearr(x8_d[:, g1])[:, 2:KT, :])
            nc.sync.dma_start(out=wk8, in_=drearr(wk8_d))
            g0 = bass.ts(0, 512)
            nc.sync.dma_start(out=xt8[:, :, g0], in_=drearr(x8_d[:, g0]))
            nc.sync.dma_start(out=wv8, in_=drearr(wv8_d))
            nc.sync.dma_start(
                out=bq_sb, in_=bq_d.rearrange("(m p o) -> p m o", p=128, o=1))
            nc.scalar.dma_start(out=xtr8[:, :, g1], in_=drearr(xr8_d[:, g1]))
            nc.scalar.dma_start(out=wqr, in_=drearr(wqr_d))
            nc.scalar.dma_start(out=wkr, in_=drearr(wkr_d))
            nc.scalar.dma_start(out=xtr8[:, :, g0], in_=drearr(xr8_d[:, g0]))
            nc.scalar.dma_start(out=wvr, in_=drearr(wvr_d))
            for g in (2, 3):
                ts_g = bass.ts(g, 512)
                nc.sync.dma_start(out=xt8[:, :, ts_g], in_=drearr(x8_d[:, ts_g]))
                nc.scalar.dma_start(out=xtr8[:, :, ts_g], in_=drearr(xr8_d[:, ts_g]))
            nc.scalar.dma_start(
                out=wp_sb, in_=wp_d.rearrange("(m p) c -> p m c", p=128))

            # ---- projection helper: 3-term residual fp8 DoubleRow ----
            def mm3(out_ap, terms, transposed):
                """accumulate 3 (stationary, moving) fp8 term pairs, K=1024"""
                n = len(terms)
                for ti, (lt, rt) in enumerate(terms):
                    for u in range(KT // 2):
                        nc.tensor.matmul(
                            out_ap, lt(u), rt(u),
                            start=(ti == 0 and u == 0),
                            stop=(ti == n - 1 and u == KT // 2 - 1),
                            perf_mode=DR,
                        )

            def unit_q(m, g):
                def emit():
                    ts_g = bass.ts(g, 512)
                    ts_m = bass.ts(m, 128)
                    qp = acc_tile()
                    mm3(qp, [
                        (lambda u: wq8[:, 2*u:2*u+2, ts_m],
                         lambda u: xt8[:, 2*u:2*u+2, ts_g]),
                        (lambda u: wqr[:, 2*u:2*u+2, ts_m],
                         lambda u: xt8[:, 2*u:2*u+2, ts_g]),
                        (lambda u: wq8[:, 2*u:2*u+2, ts_m],
                         lambda u: xtr8[:, 2*u:2*u+2, ts_g]),
                    ], False)
                    nc.scalar.activation(
                        out=qt[:, m, ts_g], in_=qp, func=ident_fn,
                        bias=bq_sb[:, m, :], scale=1.0 / WS,
                    )
                return emit

            def unit_k(m, g):
                def emit():
                    ts_g = bass.ts(g, 512)
                    ts_m = bass.ts(m, 128)
                    kp = acc_tile()
                    mm3(kp, [
                        (lambda u: wk8[:, 2*u:2*u+2, ts_m],
                         lambda u: xt8[:, 2*u:2*u+2, ts_g]),
                        (lambda u: wkr[:, 2*u:2*u+2, ts_m],
                         lambda u: xt8[:, 2*u:2*u+2, ts_g]),
                        (lambda u: wk8[:, 2*u:2*u+2, ts_m],
                         lambda u: xtr8[:, 2*u:2*u+2, ts_g]),
                    ], False)
                    nc.vector.tensor_scalar(
                        out=kt[:, m, ts_g], in0=kp,
                        scalar1=1.0 / WS, scalar2=None,
                        op0=mybir.AluOpType.mult,
                    )
                return emit

            def unit_v(it):
                def emit():
                    ts_t = bass.ts(it, 128)
                    vp_full = acc_tile()
                    vp = vp_full[:, 0:DC]
                    mm3(vp, [
                        (lambda u: xt8[:, 2*u:2*u+2, ts_t],
                         lambda u: wv8[:, 2*u:2*u+2, :]),
                        (lambda u: xt8[:, 2*u:2*u+2, ts_t],
                         lambda u: wvr[:, 2*u:2*u+2, :]),
                        (lambda u: xtr8[:, 2*u:2*u+2, ts_t],
                         lambda u: wv8[:, 2*u:2*u+2, :]),
                    ], True)
                    nc.vector.tensor_scalar(
                        out=va[:, :, it, 0:64],
                        in0=vp.rearrange("p (h d) -> p h d", h=HC),
                        scalar1=1.0 / WS, scalar2=None,
                        op0=mybir.AluOpType.mult,
                    )
                return emit

            def unit_t(hp, g):
                def emit():
                    tp = acc_tile()
                    tpb = tp.bitcast(bf16)
                    for i in range(4):
                        nc.tensor.transpose(
                            tpb[:, bass.ts(i, 128)],
                            yt_nat[:, hp, 4 * g + i, :], ident)
                    nc.vector.tensor_copy(
                        out=ytT[:, hp, bass.ts(g, 512)], in_=tpb[:, 0:512])
                return emit

            def unit_e(g, mo, deep=False):
                def emit():
                    if deep:
                        opf = ps_s.tile([128, 1024], f32, name="s12")
                        op = opf[:, 0:512]
                    else:
                        op = acc_tile()
                    for m in range(2):
                        nc.tensor.matmul(
                            op, wp_sb[:, m, bass.ts(mo, 128)],
                            ytT[:, m, bass.ts(g, 512)],
                            start=(m == 0), stop=(m == 1),
                        )
                    ob = obp.tile([128, 512], bf16, name="ob")
                    if mo % 4 == 3:
                        nc.scalar.copy(out=ob, in_=op)
                    else:
                        nc.vector.tensor_copy(out=ob, in_=op)
                    nc.sync.dma_start(
                        out=out_d[bass.ts(mo, 128), bass.ts(g, 512)], in_=ob)
                return emit

            # ---- attention chunk ----
            def attn(hp, g, pull=None):
                nj = 4 * g + 4
                dbg = debug_dumps and hp == 0 and g == 1
                yd0 = ps_y.tile([128, 4, 65], f32, name="yd")
                yd1 = ps_y.tile([128, 4, 65], f32, name="yd")
                yds = (yd0, yd1)

                def emit_s(j):
                    r = j - 4 * g
                    lo = 128 * r if r > 0 else 0
                    w = 512 - lo
                    s12 = ps_s.tile([128, 1024], f32, name="s12")
                    tsl = bass.ds(512 * g + lo, w)
                    nc.tensor.matmul(
                        s12[:, lo:512], kt[0:64, hp, bass.ts(j, 128)],
                        qt[0:64, hp, tsl], start=True, stop=True,
                    )
                    nc.tensor.matmul(
                        s12[:, 512 + lo:1024], kt[64:128, hp, bass.ts(j, 128)],
                        qt[64:128, hp, tsl], start=True, stop=True,
                    )
                    p12 = pp.tile([128, 1024], bf16, name="p12")
                    sv = s12.rearrange("p (h t) -> p h t", h=2)[:, :, lo:]
                    pv_ = p12.rearrange("p (h t) -> p h t", h=2)[:, :, lo:]
                    nc.scalar.activation(
                        out=pv_, in_=sv,
                        func=mybir.ActivationFunctionType.Exp,
                        scale=float(SCALE),
                    )
                    if r >= 0:
                        nc.gpsimd.tensor_mul(
                            p12[:, lo:lo + 128], p12[:, lo:lo + 128], tri01)
                        nc.gpsimd.tensor_mul(
                            p12[:, 512 + lo:512 + lo + 128],
                            p12[:, 512 + lo:512 + lo + 128], tri01)
                    if dbg and j <= 1:
                        d = nc.dram_tensor(
                            f"dump_p12_{j}", [128, 1024], bf16,
                            kind="ExternalOutput").ap()
                        nc.sync.dma_start(out=d, in_=p12)
                        d2 = nc.dram_tensor(
                            f"dump_s12_{j}", [128, 1024], f32,
                            kind="ExternalOutput").ap()
                        sv2 = cpool.tile([128, 1024], f32, name=f"dbg_s{j}")
                        nc.vector.tensor_copy(out=sv2, in_=s12)
                        nc.sync.dma_start(out=d2, in_=sv2)
                    return (j, p12, r)

                def emit_pv(j, p12, r):
                    # PSUM start_tensor_calc marks the whole 2KB bank pending-
                    # zero: only the FIRST matmul touching each yd bank may set
                    # start=True; the other t-subtile regions auto-zero on
                    # their first touch via the bank-wide pending mark.
                    for head in (0, 1):
                        yd = yds[head]
                        for i in range(max(r, 0), 4):
                            nc.tensor.matmul(
                                yd[:, i, :],
                                p12[:, head * 512 + 128 * i:
                                    head * 512 + 128 * (i + 1)],
                                va[:, 2 * hp + head, j, :],
                                start=(j == 0 and i == 0),
                                stop=(j == 4 * g + i),
                                skip_group_check=True,
                            )

                pend = []
                for j in range(nj):
                    pend.append(emit_s(j))
                    if len(pend) > 2:
                        emit_pv(*pend.pop(0))
                    if pull is not None and j % FILL_EVERY == FILL_EVERY - 1:
                        pull(1)
                while pend:
                    emit_pv(*pend.pop(0))

                if dbg:
                    for nm, yd in (("yd0", yd0), ("yd1", yd1)):
                        d = nc.dram_tensor(
                            f"dump_{nm}", [128, 4, 65], f32,
                            kind="ExternalOutput").ap()
                        sv = cpool.tile([128, 4, 65], f32, name=f"dbg_{nm}")
                        nc.vector.tensor_copy(out=sv, in_=yd)
                        nc.sync.dma_start(out=d, in_=sv)

                # normalize: per head one strided reciprocal [128, 4], then
                # per t-subtile a tensor_scalar multiply into yt_nat
                for head in (0, 1):
                    yd = yds[head]
                    rcp = rcpp.tile([128, 4], f32, name="rcp")
                    nc.vector.reciprocal(
                        out=rcp.rearrange("p (f o) -> p f o", o=1),
                        in_=yd[:, :, 64:65])
                    for i in range(4):
                        nc.vector.tensor_scalar(
                            out=yt_nat[:, hp, 4 * g + i,
                                       head * 64:(head + 1) * 64],
                            in0=yd[:, i, 0:64],
                            scalar1=rcp[:, i:i + 1], scalar2=None,
                            op0=mybir.AluOpType.mult,
                        )

            # ---- emission schedule (chunk order 1, 2, 3, 0) ----
            queue = []
            emitted = set()

            def push(label, fn):
                queue.append((label, fn))

            def pull(n):
                for _ in range(n):
                    if not queue:
                        return
                    lab, fn = queue.pop(0)
                    fn()
                    emitted.add(lab)

            def drain_until(label):
                if label in emitted:
                    return
                while queue:
                    lab, fn = queue.pop(0)
                    fn()
                    emitted.add(lab)
                    if lab == label:
                        return

            # K/V cover s-blocks 0..g for attention chunk g, Q covers only
            # chunk g. Eager prereqs for attn(0, 1): Q(0,1), K(0,0..1), V(0..7).
            unit_q(0, 1)()
            unit_k(0, 0)()
            unit_k(0, 1)()
            for it in range(8):
                unit_v(it)()

            push("q11", unit_q(1, 1))
            push("k10", unit_k(1, 0))
            push("k11", unit_k(1, 1))
            attn(0, 1, pull)
            push("t01", unit_t(0, 1))
            push("q02", unit_q(0, 2))
            push("k02", unit_k(0, 2))
            for it in range(8, 12):
                push(f"v{it}", unit_v(it))
            drain_until("k11")
            attn(1, 1, pull)
            push("t11", unit_t(1, 1))
            push("q12", unit_q(1, 2))
            push("k12", unit_k(1, 2))
            for mo in range(8):
                push(f"e1{mo}", unit_e(1, mo))
            drain_until("v11")
            attn(0, 2, pull)
            push("t02", unit_t(0, 2))
            push("q03", unit_q(0, 3))
            push("k03", unit_k(0, 3))
            for it in range(12, 16):
                push(f"v{it}", unit_v(it))
            drain_until("k12")
            attn(1, 2, pull)
            push("t12", unit_t(1, 2))
            push("q13", unit_q(1, 3))
            push("k13", unit_k(1, 3))
            for mo in range(8):
                push(f"e2{mo}", unit_e(2, mo))
            drain_until("v15")
            attn(0, 3, pull)
            push("t03", unit_t(0, 3))
            push("q00", unit_q(0, 0))
            drain_until("k13")
            attn(1, 3, pull)
            push("t13", unit_t(1, 3))
            push("q10", unit_q(1, 0))
            for mo in range(8):
                push(f"e3{mo}", unit_e(3, mo))
            drain_until("q00")
            attn(0, 0, pull)
            push("t00", unit_t(0, 0))
            drain_until("q10")
            attn(1, 0, pull)
            push("t10", unit_t(1, 0))
            for mo in range(8):
                push(f"e0{mo}", unit_e(0, mo, deep=True))
            pull(len(queue))

            if debug_dumps:
                for nm, t in (("qt", qt), ("kt", kt), ("va", va),
                              ("yt_nat", yt_nat), ("ytT", ytT)):
                    d = nc.dram_tensor(
                        f"dump_{nm}", list(t.shape), t.dtype,
                        kind="ExternalOutput").ap()
                    nc.sync.dma_start(out=d, in_=t)

    _split_excess_waits(nc)
    return nc


def kernel(**inputs) -> np.ndarray:
    query = np.ascontiguousarray(np.asarray(inputs["query"], dtype=np.float32))
    Wq = np.asarray(inputs["Wq"], dtype=np.float32)
    Wk = np.asarray(inputs["Wk"], dtype=np.float32)
    Wv = np.asarray(inputs["Wv"], dtype=np.float32)
    Wp = np.asarray(inputs["Wp"], dtype=np.float32)
    bq = np.asarray(inputs["bq"], dtype=np.float32)
    bk = np.asarray(inputs["bk"], dtype=np.float32)  # noqa: F841 (exactly dropped)
    bv = np.asarray(inputs["bv"], dtype=np.float32)
    bp = np.asarray(inputs["bp"], dtype=np.float32)
    n_head = int(inputs.get("n_head", H))
    assert n_head == H, f"kernel hardcodes n_head={H}, got {n_head}"
    assert query.shape == (B, T, C)

    if "nc" not in _CACHE:
        _CACHE["nc"] = _build_program()
    nc = _CACHE["nc"]

    # bv contributes bv @ Wp to every output row (softmax rows sum to 1)
    bp_eff = bp + bv @ Wp

    def split8(a):
        a8 = a.astype(F8)
        ar = (a - a8.astype(np.float32)).astype(F8)
        return a8, ar

    xb = []
    for b in range(B):
        xb.append(split8(np.ascontiguousarray(query[b].T)))
    in_maps = []
    for c in range(8):
        b, hg = divmod(c, 4)
        cols = slice(DC * hg, DC * (hg + 1))
        wq8, wqr = split8(WS * np.ascontiguousarray(Wq[:, cols]))
        wk8, wkr = split8(WS * np.ascontiguousarray(Wk[:, cols]))
        wv8, wvr = split8(WS * np.ascontiguousarray(Wv[:, cols]))
        in_maps.append({
            "x8": xb[b][0], "xr8": xb[b][1],
            "wq8": wq8, "wqr": wqr,
            "wk8": wk8, "wkr": wkr,
            "wv8": wv8, "wvr": wvr,
            "wp": np.ascontiguousarray(Wp[cols, :]).astype(BF),
            "bq": np.ascontiguousarray(bq[cols]),
        })

    res = run_bass_kernel_spmd(nc, in_maps, core_ids=list(range(8)))
    _CACHE["last_res"] = res

    out = np.empty((B, T, C), np.float32)
    for b in range(B):
        acc = np.asarray(res.results[4 * b]["out_t"], dtype=np.float32)
        for c in range(4 * b + 1, 4 * b + 4):
            acc = acc + np.asarray(res.results[c]["out_t"], dtype=np.float32)
        out[b] = acc.T + bp_eff
    return out


# revision 35
# speedup vs baseline: 1.2263x; 1.1043x over previous
"""Causal self-attention Bass/TRN2 kernel (v3).

Shapes (hardcoded): query [2, 2048, 1024], 16 heads, d=64.
Sharding: 8 cores = 2 batches x 4 head-groups (4 heads per core, tensor
parallel on the QKV/proj weight columns). Each core computes a partial
out projection out_t = Wp_slice^T @ y^T (shape [1024, 2048] bf16); host sums
the 4 partials per batch, transposes, and adds the folded bias.

Cost-model-driven structure (TimelineSim charges moving-columns only):
  * QKV projections run as 3-term residual-fp8 DoubleRow matmuls
    (x8@w8 + x8@wr + xr@w8, all e4m3, K=256 per instruction at 0.5
    cycles/row): 25% fewer PE cycles than bf16 at slightly BETTER accuracy
    (x8+xr carries ~9 mantissa bits vs bf16's 8). Weights are pre-scaled
    by 32 on host so all three terms share one PSUM scale; the PSUM->SBUF
    copy divides by 32.
  * PV uses p12 as the STATIONARY operand and the V-block [128, 65]
    (64 dims + ones column for the denominator) as MOVING: out y lands in
    natural [t, d] layout at 65 moving-columns per (j-block, t-subtile)
    instead of w columns -- ~2x cheaper than the y^T formulation.
  * The softmax denominator is then a per-partition scalar: DVE reciprocal
    [128, 4] + tensor_scalar multiply replace the reciprocal/broadcast-
    matmul/rowcopy chain of the y^T formulation.
  * y^T for the out-projection is recovered with PE transposes (128 cycles
    per [128, 128] tile; identity-permutation matmul).
  * bk dropped (softmax shift invariance); bv folded into the output bias
    on host; bq applied in the Q PSUM->SBUF copy.

Per-core pipeline:
  B(m,g): Q^T/K^T [128, 512]-chunk projections (12 DoubleRow matmuls) ->
          qt/kt bf16. Q copy on ACT (bias+1/32 scale), K on DVE.
  C(it):  V natural [128, 256] (12 DoubleRow matmuls) + DVE scaled copy into
          va bf16 [128, h, it, 65]; column 64 memset to 1.
  D(hp,g): per 128-row K-block j: S^T for both heads into one [128, 1024]
          PSUM tile; ACT exp (scale=1/8) -> p12 bf16; Pool tri01 mask on the
          diagonal blocks; PV accumulates into yd [128, 4, 65] natural tiles
          (one per t-subtile per head). Emission software-pipelined 2 blocks
          ahead; projection/transpose/out-proj units pulled as PE filler.
          Normalize: DVE reciprocal of yd[:, :, 64] + tensor_scalar -> yt_nat.
  T(hp,g): 4 PE transposes [128, 128] -> one DVE copy into ytT.
  E(g,mo): out_t chunk = Wp^T y^T (bf16), staged PSUM->SBUF on DVE/ACT,
          DMA out bf16.
Chunk order 1,2,3,0 so the final attention chunk (and its trailing exp
backlog) is the smallest.

Walrus accepts only ONE sync-wait per TPB instruction; excess waits are
hoisted into standalone InstEventSemaphore (split_excess_waits).
"""

import numpy as np
import ml_dtypes

import concourse.bass as bass
import concourse.mybir as mybir
import concourse.tile as tile
from concourse.bass_utils import run_bass_kernel_spmd

B, T, C, H = 2, 2048, 1024, 16
D = C // H            # 64 head dim
HC = 4                # heads per core
DC = HC * D           # 256 dcols per core
KT = C // 128         # 8 contraction tiles
NT = T // 128         # 16 t-tiles
TCH = T // 512        # 4 t-chunks of 512
SCALE = 1.0 / np.sqrt(D)
FILL_EVERY = 2
PEND = 4
WS = 32.0             # host weight pre-scale for fp8

f32 = mybir.dt.float32
bf16 = mybir.dt.bfloat16
fp8 = mybir.dt.float8e4
BF = ml_dtypes.bfloat16
F8 = ml_dtypes.float8_e4m3
DR = mybir.MatmulPerfMode.DoubleRow

_CACHE = {}


def _split_excess_waits(nc, max_inline=1):
    """Hoist excess per-instruction waits into standalone event-sem waits."""
    n = 0
    for f in nc.m.functions:
        for bb in f.blocks:
            new_insts = []
            for inst in bb.instructions:
                si = inst.sync_info
                waits = list(si.on_wait) if (si is not None and si.on_wait) else []
                if len(waits) > max_inline:
                    hoist, keep = waits[:-max_inline], waits[-max_inline:]
                    for w in hoist:
                        ev = mybir.InstEventSemaphore(
                            name=nc.get_next_instruction_name(),
                            engine=inst.engine,
                            ins=[],
                            outs=[],
                            sync_info=mybir.SyncInfo(on_wait=[w], on_update=[]),
                        )
                        nc.register_instruction(ev, overwrite=True)
                        new_insts.append(ev)
                        n += 1
                    si.on_wait = keep
                new_insts.append(inst)
            bb.instructions[:] = new_insts
    return n


def _build_program(debug_dumps=False):
    nc = bass.Bass("TRN2", target_bir_lowering=False, debug=False)

    x8_d = nc.dram_tensor("x8", [C, T], fp8, kind="ExternalInput").ap()
    xr8_d = nc.dram_tensor("xr8", [C, T], fp8, kind="ExternalInput").ap()
    # weight main/residual pairs packed [C, 2, DC] so each DMA row is 512B
    # contiguous (fp8 rows of 256B pay a 2x DMA latency penalty)
    wqp_d = nc.dram_tensor("wqp", [C, 2, DC], fp8, kind="ExternalInput").ap()
    wkp_d = nc.dram_tensor("wkp", [C, 2, DC], fp8, kind="ExternalInput").ap()
    wvp_d = nc.dram_tensor("wvp", [C, 2, DC], fp8, kind="ExternalInput").ap()
    wp_d = nc.dram_tensor("wp", [DC, C], bf16, kind="ExternalInput").ap()
    bq_d = nc.dram_tensor("bq", [DC], f32, kind="ExternalInput").ap()
    out_d = nc.dram_tensor("out_t", [C, T], bf16, kind="ExternalOutput").ap()

    ident_fn = mybir.ActivationFunctionType.Identity

    def drearr(ap2d):
        return ap2d.rearrange("(k p) t -> p k t", p=128)

    with (
        tile.TileContext(nc) as tc,
        nc.allow_low_precision("fp8/bf16 compute fits the 2e-2 rel tolerance"),
    ):
        with (
            tc.tile_pool(name="const", bufs=1) as cpool,
            tc.tile_pool(name="big", bufs=1) as big,
            tc.tile_pool(name="pp", bufs=12) as pp,
            tc.tile_pool(name="rcpp", bufs=6) as rcpp,
            tc.tile_pool(name="obp", bufs=8) as obp,
            tc.tile_pool(name="ps_a", bufs=2, space="PSUM") as ps_a,
            tc.tile_pool(name="ps_s", bufs=2, space="PSUM") as ps_s,
            tc.tile_pool(name="ps_y", bufs=2, space="PSUM") as ps_y,
        ):
            # PSUM budget (8 banks): acc 2x1, s12 2x2, yd 2x1.
            def acc_tile():
                return ps_a.tile([128, 512], f32, name="acc")

            # ---- constants ----
            # tri01[p, f] = 1 where f >= p (valid, t>=s) else 0 (Pool mask)
            tri01 = cpool.tile([128, 128], bf16)
            nc.gpsimd.memset(tri01, 1.0)
            nc.gpsimd.affine_select(
                out=tri01, in_=tri01, compare_op=mybir.AluOpType.is_ge,
                fill=0.0, base=0, pattern=[[1, 128]], channel_multiplier=-1,
            )
            # identity permutation for PE transposes
            ident = cpool.tile([128, 128], bf16)
            nc.gpsimd.memset(ident, 1.0)
            nc.gpsimd.affine_select(
                out=ident, in_=ident, compare_op=mybir.AluOpType.is_equal,
                fill=0.0, base=0, pattern=[[1, 128]], channel_multiplier=-1,
            )
            bq_sb = cpool.tile([128, 2, 1], f32)

            # ---- persistent big tensors ----
            xt8 = big.tile([128, KT, T], fp8)
            xtr8 = big.tile([128, KT, T], fp8)
            wqp = big.tile([128, KT, 2, DC], fp8)
            wkp = big.tile([128, KT, 2, DC], fp8)
            wvp = big.tile([128, KT, 2, DC], fp8)
            wp_sb = big.tile([128, 2, C], bf16)
            qt = big.tile([128, 2, T], bf16)       # Q^T [dcol, t]
            kt = big.tile([128, 2, T], bf16)       # K^T
            # V augmented per head: [s, 65] = [V_h | ones]
            va = big.tile([128, HC, NT, 65], bf16)
            yt_nat = big.tile([128, 2, NT, 128], bf16)  # y natural [t, dc]
            ytT = big.tile([128, 2, T], bf16)           # y^T [dc, t]

            nc.gpsimd.memset(va[:, :, :, 64:65], 1.0)

            # ---- input DMAs, ordered for earliest PE start ----
            # first attention chunk is g=0: needs wq/wk (+residuals) and the
            # g0 x slices only.
            g0 = bass.ts(0, 512)

            def wrearr(ap3d):
                return ap3d.rearrange("(k p) s d -> p k s d", p=128)

            nc.sync.dma_start(out=wqp, in_=wrearr(wqp_d))
            nc.sync.dma_start(out=xt8[:, 0:2, g0], in_=drearr(x8_d[:, g0])[:, 0:2, :])
            nc.sync.dma_start(out=xt8[:, 2:KT, g0], in_=drearr(x8_d[:, g0])[:, 2:KT, :])
            nc.sync.dma_start(out=wkp, in_=wrearr(wkp_d))
            nc.scalar.dma_start(out=xtr8[:, :, g0], in_=drearr(xr8_d[:, g0]))
            nc.sync.dma_start(out=wvp, in_=wrearr(wvp_d))
            nc.sync.dma_start(
                out=bq_sb, in_=bq_d.rearrange("(m p o) -> p m o", p=128, o=1))
            for g in (1, 2, 3):
                ts_g = bass.ts(g, 512)
                nc.sync.dma_start(out=xt8[:, :, ts_g], in_=drearr(x8_d[:, ts_g]))
                nc.scalar.dma_start(out=xtr8[:, :, ts_g], in_=drearr(xr8_d[:, ts_g]))
                if g == 1:
                    nc.scalar.dma_start(
                        out=wp_sb, in_=wp_d.rearrange("(m p) c -> p m c", p=128))

            # ---- projection helper: 3-term residual fp8 DoubleRow ----
            def mm3(out_ap, terms, transposed):
                """accumulate 3 (stationary, moving) fp8 term pairs, K=1024"""
                n = len(terms)
                for ti, (lt, rt) in enumerate(terms):
                    for u in range(KT // 2):
                        nc.tensor.matmul(
                            out_ap, lt(u), rt(u),
                            start=(ti == 0 and u == 0),
                            stop=(ti == n - 1 and u == KT // 2 - 1),
                            perf_mode=DR,
                        )

            def unit_q(m, g):
                def emit():
                    ts_g = bass.ts(g, 512)
                    ts_m = bass.ts(m, 128)
                    qp = acc_tile()
                    mm3(qp, [
                        (lambda u: wqp[:, 2*u:2*u+2, 0, ts_m],
                         lambda u: xt8[:, 2*u:2*u+2, ts_g]),
                        (lambda u: wqp[:, 2*u:2*u+2, 1, ts_m],
                         lambda u: xt8[:, 2*u:2*u+2, ts_g]),
                        (lambda u: wqp[:, 2*u:2*u+2, 0, ts_m],
                         lambda u: xtr8[:, 2*u:2*u+2, ts_g]),
                    ], False)
                    nc.vector.tensor_scalar(
                        out=qt[:, m, ts_g], in0=qp,
                        scalar1=1.0 / WS, scalar2=bq_sb[:, m, :],
                        op0=mybir.AluOpType.mult, op1=mybir.AluOpType.add,
                    )
                return emit

            def unit_k(m, g):
                def emit():
                    ts_g = bass.ts(g, 512)
                    ts_m = bass.ts(m, 128)
                    kp = acc_tile()
                    mm3(kp, [
                        (lambda u: wkp[:, 2*u:2*u+2, 0, ts_m],
                         lambda u: xt8[:, 2*u:2*u+2, ts_g]),
                        (lambda u: wkp[:, 2*u:2*u+2, 1, ts_m],
                         lambda u: xt8[:, 2*u:2*u+2, ts_g]),
                        (lambda u: wkp[:, 2*u:2*u+2, 0, ts_m],
                         lambda u: xtr8[:, 2*u:2*u+2, ts_g]),
                    ], False)
                    nc.vector.tensor_scalar(
                        out=kt[:, m, ts_g], in0=kp,
                        scalar1=1.0 / WS, scalar2=None,
                        op0=mybir.AluOpType.mult,
                    )
                return emit

            def unit_v(it):
                def emit():
                    ts_t = bass.ts(it, 128)
                    vp_full = acc_tile()
                    vp = vp_full[:, 0:DC]
                    mm3(vp, [
                        (lambda u: xt8[:, 2*u:2*u+2, ts_t],
                         lambda u: wvp[:, 2*u:2*u+2, 0, :]),
                        (lambda u: xt8[:, 2*u:2*u+2, ts_t],
                         lambda u: wvp[:, 2*u:2*u+2, 1, :]),
                        (lambda u: xtr8[:, 2*u:2*u+2, ts_t],
                         lambda u: wvp[:, 2*u:2*u+2, 0, :]),
                    ], True)
                    nc.vector.tensor_scalar(
                        out=va[:, :, it, 0:64],
                        in0=vp.rearrange("p (h d) -> p h d", h=HC),
                        scalar1=1.0 / WS, scalar2=None,
                        op0=mybir.AluOpType.mult,
                    )
                return emit

            def unit_t(hp, g):
                def emit():
                    tp = acc_tile()
                    tpb = tp.bitcast(bf16)
                    for i in range(4):
                        nc.tensor.transpose(
                            tpb[:, bass.ts(i, 128)],
                            yt_nat[:, hp, 4 * g + i, :], ident)
                    nc.vector.tensor_copy(
                        out=ytT[:, hp, bass.ts(g, 512)], in_=tpb[:, 0:512])
                return emit

            def unit_e(g, mo, deep=False):
                def emit():
                    if deep:
                        opf = ps_s.tile([128, 1024], f32, name="s12")
                        op = opf[:, 0:512]
                    else:
                        op = acc_tile()
                    for m in range(2):
                        nc.tensor.matmul(
                            op, wp_sb[:, m, bass.ts(mo, 128)],
                            ytT[:, m, bass.ts(g, 512)],
                            start=(m == 0), stop=(m == 1),
                        )
                    ob = obp.tile([128, 512], bf16, name="ob")
                    if (mo % 2 == 1) if deep else (mo % 4 == 3):
                        nc.scalar.copy(out=ob, in_=op)
                    else:
                        nc.vector.tensor_copy(out=ob, in_=op)
                    q = nc.scalar if mo % 2 else nc.sync
                    q.dma_start(
                        out=out_d[bass.ts(mo, 128), bass.ts(g, 512)], in_=ob)
                return emit

            # ---- attention chunk ----
            def attn(hp, g, pull=None):
                nj = 4 * g + 4
                dbg = debug_dumps and hp == 0 and g == 1
                yd0 = ps_y.tile([128, 4, 65], f32, name="yd")
                yd1 = ps_y.tile([128, 4, 65], f32, name="yd")
                yds = (yd0, yd1)

                def emit_s(j):
                    r = j - 4 * g
                    lo = 128 * r if r > 0 else 0
                    w = 512 - lo
                    s12 = ps_s.tile([128, 1024], f32, name="s12")
                    tsl = bass.ds(512 * g + lo, w)
                    nc.tensor.matmul(
                        s12[:, lo:512], kt[0:64, hp, bass.ts(j, 128)],
                        qt[0:64, hp, tsl], start=True, stop=True,
                    )
                    nc.tensor.matmul(
                        s12[:, 512 + lo:1024], kt[64:128, hp, bass.ts(j, 128)],
                        qt[64:128, hp, tsl], start=True, stop=True,
                    )
                    p12 = pp.tile([128, 1024], bf16, name="p12")
                    sv = s12.rearrange("p (h t) -> p h t", h=2)[:, :, lo:]
                    pv_ = p12.rearrange("p (h t) -> p h t", h=2)[:, :, lo:]
                    nc.scalar.activation(
                        out=pv_, in_=sv,
                        func=mybir.ActivationFunctionType.Exp,
                        scale=float(SCALE),
                    )
                    if r >= 0:
                        nc.gpsimd.tensor_mul(
                            p12[:, lo:lo + 128], p12[:, lo:lo + 128], tri01)
                        nc.gpsimd.tensor_mul(
                            p12[:, 512 + lo:512 + lo + 128],
                            p12[:, 512 + lo:512 + lo + 128], tri01)
                    if dbg and j <= 1:
                        d = nc.dram_tensor(
                            f"dump_p12_{j}", [128, 1024], bf16,
                            kind="ExternalOutput").ap()
                        nc.sync.dma_start(out=d, in_=p12)
                        d2 = nc.dram_tensor(
                            f"dump_s12_{j}", [128, 1024], f32,
                            kind="ExternalOutput").ap()
                        sv2 = cpool.tile([128, 1024], f32, name=f"dbg_s{j}")
                        nc.vector.tensor_copy(out=sv2, in_=s12)
                        nc.sync.dma_start(out=d2, in_=sv2)
                    return (j, p12, r)

                def emit_pv(j, p12, r):
                    # PSUM start_tensor_calc marks the whole 2KB bank pending-
                    # zero: only the FIRST matmul touching each yd bank may set
                    # start=True; the other t-subtile regions auto-zero on
                    # their first touch via the bank-wide pending mark.
                    for head in (0, 1):
                        yd = yds[head]
                        for i in range(max(r, 0), 4):
                            nc.tensor.matmul(
                                yd[:, i, :],
                                p12[:, head * 512 + 128 * i:
                                    head * 512 + 128 * (i + 1)],
                                va[:, 2 * hp + head, j, :],
                                start=(j == 0 and i == 0),
                                stop=(j == 4 * g + i),
                                skip_group_check=True,
                            )

                pend = []
                for j in range(nj):
                    pend.append(emit_s(j))
                    if len(pend) > PEND:
                        emit_pv(*pend.pop(0))
                    if pull is not None and j % FILL_EVERY == FILL_EVERY - 1:
                        pull(1)
                while pend:
                    emit_pv(*pend.pop(0))

                if dbg:
                    for nm, yd in (("yd0", yd0), ("yd1", yd1)):
                        d = nc.dram_tensor(
                            f"dump_{nm}", [128, 4, 65], f32,
                            kind="ExternalOutput").ap()
                        sv = cpool.tile([128, 4, 65], f32, name=f"dbg_{nm}")
                        nc.vector.tensor_copy(out=sv, in_=yd)
                        nc.sync.dma_start(out=d, in_=sv)

                # normalize: per head one strided reciprocal [128, 4], then
                # per t-subtile a tensor_scalar multiply into yt_nat
                for head in (0, 1):
                    yd = yds[head]
                    rcp = rcpp.tile([128, 4], f32, name="rcp")
                    nc.vector.reciprocal(
                        out=rcp.rearrange("p (f o) -> p f o", o=1),
                        in_=yd[:, :, 64:65])
                    for i in range(4):
                        nc.vector.tensor_scalar(
                            out=yt_nat[:, hp, 4 * g + i,
                                       head * 64:(head + 1) * 64],
                            in0=yd[:, i, 0:64],
                            scalar1=rcp[:, i:i + 1], scalar2=None,
                            op0=mybir.AluOpType.mult,
                        )

            # ---- emission schedule (chunk order 1, 2, 3, 0) ----
            queue = []
            emitted = set()

            def push(label, fn):
                queue.append((label, fn))

            def pull(n):
                for _ in range(n):
                    if not queue:
                        return
                    lab, fn = queue.pop(0)
                    fn()
                    emitted.add(lab)

            def drain_until(label):
                if label in emitted:
                    return
                while queue:
                    lab, fn = queue.pop(0)
                    fn()
                    emitted.add(lab)
                    if lab == label:
                        return

            # K/V cover s-blocks 0..g for attention chunk g, Q covers only
            # chunk g. Chunk order 0, 2, 3, 1: the first chunk needs only the
            # g0 input slices (fastest PE start) and the last chunk is small
            # (short trailing exp backlog before the final out-projection).
            unit_q(0, 0)()
            unit_k(0, 0)()
            for it in range(4):
                unit_v(it)()

            def pull2(n):
                pull(2 * n)

            push("q10", unit_q(1, 0))
            push("k10", unit_k(1, 0))
            attn(0, 0, pull)
            push("t00", unit_t(0, 0))
            push("k01", unit_k(0, 1))
            push("q02", unit_q(0, 2))
            push("k02", unit_k(0, 2))
            for it in range(4, 12):
                push(f"v{it}", unit_v(it))
            drain_until("k10")
            attn(1, 0, pull)
            push("t10", unit_t(1, 0))
            push("q12", unit_q(1, 2))
            push("k11", unit_k(1, 1))
            push("k12", unit_k(1, 2))
            for mo in range(8):
                push(f"e0{mo}", unit_e(0, mo))
            drain_until("v11")
            attn(0, 2, pull)
            push("t02", unit_t(0, 2))
            push("q03", unit_q(0, 3))
            push("k03", unit_k(0, 3))
            for it in range(12, 16):
                push(f"v{it}", unit_v(it))
            drain_until("k12")
            attn(1, 2, pull)
            push("t12", unit_t(1, 2))
            push("q13", unit_q(1, 3))
            push("k13", unit_k(1, 3))
            for mo in range(8):
                push(f"e2{mo}", unit_e(2, mo))
            drain_until("v15")
            attn(0, 3, pull)
            push("t03", unit_t(0, 3))
            push("q01", unit_q(0, 1))
            drain_until("k13")
            attn(1, 3, pull)
            push("t13", unit_t(1, 3))
            push("q11", unit_q(1, 1))
            for mo in range(8):
                push(f"e3{mo}", unit_e(3, mo))
            drain_until("q01")
            attn(0, 1, pull)
            push("t01", unit_t(0, 1))
            drain_until("q11")
            attn(1, 1, pull)
            push("t11", unit_t(1, 1))
            for mo in range(8):
                push(f"e1{mo}", unit_e(1, mo, deep=True))
            pull(len(queue))

            if debug_dumps:
                for nm, t in (("qt", qt), ("kt", kt), ("va", va),
                              ("yt_nat", yt_nat), ("ytT", ytT)):
                    d = nc.dram_tensor(
                        f"dump_{nm}", list(t.shape), t.dtype,
                        kind="ExternalOutput").ap()
                    nc.sync.dma_start(out=d, in_=t)

    _split_excess_waits(nc)
    return nc


def kernel(**inputs) -> np.ndarray:
    query = np.ascontiguousarray(np.asarray(inputs["query"], dtype=np.float32))
    Wq = np.asarray(inputs["Wq"], dtype=np.float32)
    Wk = np.asarray(inputs["Wk"], dtype=np.float32)
    Wv = np.asarray(inputs["Wv"], dtype=np.float32)
    Wp = np.asarray(inputs["Wp"], dtype=np.float32)
    bq = np.asarray(inputs["bq"], dtype=np.float32)
    bk = np.asarray(inputs["bk"], dtype=np.float32)  # noqa: F841 (exactly dropped)
    bv = np.asarray(inputs["bv"], dtype=np.float32)
    bp = np.asarray(inputs["bp"], dtype=np.float32)
    n_head = int(inputs.get("n_head", H))
    assert n_head == H, f"kernel hardcodes n_head={H}, got {n_head}"
    assert query.shape == (B, T, C)

    if "nc" not in _CACHE:
        _CACHE["nc"] = _build_program()
    nc = _CACHE["nc"]

    # bv contributes bv @ Wp to every output row (softmax rows sum to 1)
    bp_eff = bp + bv @ Wp

    def split8(a):
        a8 = a.astype(F8)
        ar = (a - a8.astype(np.float32)).astype(F8)
        return a8, ar

    def pack8(w):
        w8, wr = split8(WS * np.ascontiguousarray(w))
        return np.ascontiguousarray(np.stack([w8, wr], axis=1))

    xb = []
    for b in range(B):
        xb.append(split8(np.ascontiguousarray(query[b].T)))
    in_maps = []
    for c in range(8):
        b, hg = divmod(c, 4)
        cols = slice(DC * hg, DC * (hg + 1))
        in_maps.append({
            "x8": xb[b][0], "xr8": xb[b][1],
            "wqp": pack8(Wq[:, cols]),
            "wkp": pack8(Wk[:, cols]),
            "wvp": pack8(Wv[:, cols]),
            "wp": np.ascontiguousarray(Wp[cols, :]).astype(BF),
            "bq": np.ascontiguousarray(bq[cols]),
        })

    res = run_bass_kernel_spmd(nc, in_maps, core_ids=list(range(8)))
    _CACHE["last_res"] = res

    out = np.empty((B, T, C), np.float32)
    for b in range(B):
        acc = np.asarray(res.results[4 * b]["out_t"], dtype=np.float32)
        for c in range(4 * b + 1, 4 * b + 4):
            acc = acc + np.asarray(res.results[c]["out_t"], dtype=np.float32)
        out[b] = acc.T + bp_eff
    return out
